# revision 1
# baseline (speedup 1.0000x reference)
"""Trainium2 Bass kernel for nn_EncodingNet (FastGTN-style GNN).

Self-contained: the host shards/packs inputs (index bucketing + repacking
only -- no value arithmetic), builds + runs an 8-core SPMD Bass kernel via
PJRT (axon), and gathers the full output.

Algorithmic structure (operator form -- never materializes mats1 @ mats0):
  E_t = densify(edge_index[t], edge_value[t])        [2048, 2048] per type
  mats_l[c] = sum_t softmax(conv_w[l])[c,t] * E_t    (materialized per core
              as row-shards in SBUF, fp32, built from int16 scattered E)
  6 sequential row-parallel GEMM passes over mats cover GT layer 0, GT
  layer 1, GCN1 (mats0, mats1), GCN2 (mats0, mats1); an AllGather after
  each pass rebuilds the full-height RHS for the next.

Sharding: nodes row-sharded over 8 cores (256 rows/core). Edge values are
scatter-packed via gpsimd local_scatter as int16 fixed point (duplicates
are summed on-device in fp32 before quantization); the dequant scale is
folded into the softmax(conv_w) coefficients.
"""

import os
import sys
import types

import numpy as np

# ---------------------------------------------------------------------------
# Environment workaround (inline: kernel.py must be self-contained).
# ---------------------------------------------------------------------------
if "antenv.axon_hooks" not in sys.modules:
    _m = types.ModuleType("antenv.axon_hooks")
    _m.get_axon_ntff_profile_hook = lambda: None
    sys.modules["antenv.axon_hooks"] = _m

import concourse.bass as bass
import concourse.bacc as bacc
import concourse.tile as tile
from concourse import mybir

# ---------------------------------------------------------------------------
# Problem constants (hardcoded per the task contract).
# ---------------------------------------------------------------------------
N = 2048          # nodes
C = 2             # channels
T = 3             # edge types
L = 2             # GT layers
E = 65536         # edges per type
W_IN = 256
W_OUT = 64
NCLS = 16
NTGT = 512
BETA = 0.5

NCORES = 8
NS = N // NCORES  # 256 rows per core
P = 128
KC = 16           # node chunks: node j = p*16 + kc
NEL = 1024        # local_scatter num_elems per call
NCH = (T * KC * NS) // NEL  # 12 scatter chunks
EFREE = T * KC * NS         # 12288 free elems of the E^T SBUF tile
MFREE = KC * NS             # 4096 free elems of one mats tile

f32 = mybir.dt.float32
i16 = mybir.dt.int16
i32 = mybir.dt.int32
AF = mybir.ActivationFunctionType
OP = mybir.AluOpType

# misc pack offsets (columns in the [128, MISC_W] fp32 misc tensor)
_MO_CONV = 0          # [12]
_MO_B1 = 12           # [16]
_MO_B2 = 28           # [64]
_MO_LB = 92           # [16]
_MO_ID = 108          # [128]
_MO_GW1 = 236         # [16]  (partitions 0..63)
_MO_GW2 = 252         # [64]  (partitions 0..15)
_MO_LW = 316          # [16]  (partitions 0..63)
_MO_TIDX = 332        # [4]   (int32 bits)
MISC_W = 336


# ---------------------------------------------------------------------------
# Host-side packing.
# ---------------------------------------------------------------------------
def _prep_inputs(X, edge_value, conv_w, Ws, gcn_w1, gcn_b1, gcn_w2, gcn_b2,
                 lin_w, lin_b, edge_index, target_x):
    X = np.asarray(X, np.float32)
    edge_value = np.asarray(edge_value, np.float32)
    conv_w = np.asarray(conv_w, np.float32)
    Ws = np.asarray(Ws, np.float32)
    gcn_w1 = np.asarray(gcn_w1, np.float32)
    gcn_b1 = np.asarray(gcn_b1, np.float32)
    gcn_w2 = np.asarray(gcn_w2, np.float32)
    gcn_b2 = np.asarray(gcn_b2, np.float32)
    lin_w = np.asarray(lin_w, np.float32)
    lin_b = np.asarray(lin_b, np.float32)
    ei = np.asarray(edge_index, np.int64)
    tx = np.asarray(target_x, np.int64)

    # xT_perm[:, kc*128 + p] = X.T[:, p*16 + kc]  (node j = p*16 + kc)
    kk, pp = np.meshgrid(np.arange(KC), np.arange(P), indexing="ij")
    pos_node = (pp * KC + kk).reshape(-1)
    xT_perm = np.ascontiguousarray(X[pos_node].T)          # [256, 2048]
    ws_cat = np.concatenate([Ws[0], Ws[1]], axis=1)        # [256, 128]

    def fold(a):  # [256, F] -> [128, 2*F]  (feat = kf*128 + p)
        fdim = a.shape[1]
        return np.ascontiguousarray(
            a.reshape(2, P, fdim).transpose(1, 0, 2).reshape(P, 2 * fdim))

    misc = np.zeros((P, MISC_W), np.float32)
    misc[:, _MO_CONV:_MO_CONV + 12] = conv_w.reshape(1, -1)
    misc[:, _MO_B1:_MO_B1 + 16] = gcn_b1.reshape(1, -1)
    misc[:, _MO_B2:_MO_B2 + 64] = gcn_b2.reshape(1, -1)
    misc[:, _MO_LB:_MO_LB + 16] = lin_b.reshape(1, -1)
    misc[:, _MO_ID:_MO_ID + 128] = np.eye(P, dtype=np.float32)
    misc[:64, _MO_GW1:_MO_GW1 + 16] = gcn_w1
    misc[:16, _MO_GW2:_MO_GW2 + 64] = gcn_w2
    misc[:64, _MO_LW:_MO_LW + 16] = lin_w
    tidx = np.ascontiguousarray(
        tx.reshape(NTGT // P, P).T.astype(np.int32))       # [128, 4]
    misc[:, _MO_TIDX:_MO_TIDX + 4] = tidx.view(np.float32)

    # ---- edge bucketing per core (vectorized, index-only) -----------------
    t_id = np.repeat(np.arange(T, dtype=np.int64), E)
    r_all = ei[:, 0, :].reshape(-1)
    c_all = ei[:, 1, :].reshape(-1)
    v_all = edge_value.reshape(-1)
    rank = r_all >> 8
    r_loc = r_all & 255
    p_of = c_all >> 4
    kc_of = c_all & 15
    free = t_id * MFREE + kc_of * NS + r_loc
    ch_of = free // NEL
    pos_of = free % NEL
    bucket = (rank * P + p_of) * NCH + ch_of
    cell = bucket * NEL + pos_of

    order = np.argsort(cell, kind="stable")
    cell_s = cell[order]
    v_s = v_all[order]
    ucell, first_idx, counts = np.unique(cell_s, return_index=True,
                                         return_counts=True)
    occ = np.arange(len(cell_s)) - np.repeat(first_idx, counts)
    M = int(counts.max())
    ubucket = ucell // NEL
    upos = ucell % NEL
    # order unique cells within each bucket with duplicated cells FIRST so
    # the device only runs the dup-sum adds over a tiny slot window
    order2 = np.lexsort((np.arange(len(ucell)), counts == 1, ubucket))
    inv2 = np.empty_like(order2)
    inv2[order2] = np.arange(len(order2))
    ub_sorted = ubucket[order2]
    ub_uniq, ub_fidx, ub_counts = np.unique(ub_sorted, return_index=True,
                                            return_counts=True)
    slot_sorted = np.arange(len(ucell)) - np.repeat(ub_fidx, ub_counts)
    slot = slot_sorted[inv2]
    ndup_per_bucket = np.zeros(len(ub_uniq), np.int64)
    isdup_sorted = (counts[order2] >= 2)
    np.add.at(ndup_per_bucket,
              np.searchsorted(ub_uniq, ub_sorted), isdup_sorted)
    DUPW = int(ndup_per_bucket.max()) if M > 1 else 0
    max_cnt = int(ub_counts.max())
    NI = max_cnt + (max_cnt & 1)
    scale = float(2.0 ** np.floor(np.log2(32767.0 / M)))

    scat_idx = np.full((NCORES, P, NCH * NI), -1, np.int16)
    scat_vals = np.zeros((NCORES, M, P, NCH * NI), np.float32)
    uk = ubucket // (P * NCH)
    up = (ubucket // NCH) % P
    uch = ubucket % NCH
    scat_idx[uk, up, uch * NI + slot] = upos.astype(np.int16)
    cell_row = np.searchsorted(ucell, cell_s)
    scat_vals[uk[cell_row], occ, up[cell_row],
              uch[cell_row] * NI + slot[cell_row]] = v_s

    big0_shared = np.concatenate([fold(xT_perm), fold(ws_cat)], axis=1)
    in_maps = []
    for k in range(NCORES):
        xmy = fold(np.ascontiguousarray(X[k * NS:(k + 1) * NS].T))
        m = {
            "big0": np.ascontiguousarray(
                np.concatenate([big0_shared, xmy], axis=1)),
            "misc": misc,
            "sidx": scat_idx[k],
            "svals": np.ascontiguousarray(
                scat_vals[k].transpose(1, 0, 2).reshape(P, -1)),
        }
        in_maps.append(m)
    return in_maps, NI, M, scale, DUPW


# ---------------------------------------------------------------------------
# Device kernel.
# ---------------------------------------------------------------------------
class _StageStop(Exception):
    pass


def build_kernel(NI, M, scale, DUPW=0, reps=1, stop_after=None):
    nc = bacc.Bacc("TRN2", target_bir_lowering=False, debug=False,
                   num_devices=NCORES)
    F = NCH * NI
    # big0 layout: [xT fold (4096) | ws fold (256) | xmy fold (512)]
    XT_OFF = 0
    WS_OFF = 2 * N
    XMY_OFF = 2 * N + 2 * C * W_OUT
    BIG0_W = XMY_OFF + 2 * NS

    big0_d = nc.dram_tensor("big0", [P, BIG0_W], f32, kind="ExternalInput")
    misc_d = nc.dram_tensor("misc", [P, MISC_W], f32, kind="ExternalInput")
    sidx_d = nc.dram_tensor("sidx", [P, F], i16, kind="ExternalInput")
    svals_d = nc.dram_tensor("svals", [P, M * F], f32, kind="ExternalInput")
    y_d = nc.dram_tensor("y", [NTGT, NCLS], f32, kind="ExternalOutput")

    ccds = []
    for r in range(reps):
        ccd = {}
        for name, d in [("A", 130), ("C", 16), ("D", C * 16),
                        ("E", W_OUT), ("F", C * W_OUT), ("H", W_OUT)]:
            ccd[name] = (
                nc.dram_tensor(f"cci_{name}{r}", [NS, d], f32),
                nc.dram_tensor(f"cco_{name}{r}", [N, d], f32,
                               addr_space="Shared"),
                d,
            )
        ccds.append(ccd)
    rg = [list(range(NCORES))]

    with tile.TileContext(nc) as tc:
        import contextlib
        ctx = contextlib.ExitStack()
        with ctx:
            pool = ctx.enter_context(tc.tile_pool(name="main", bufs=1))
            ppool = ctx.enter_context(
                tc.tile_pool(name="pass_psum", bufs=4, space="PSUM"))
            apool = ctx.enter_context(
                tc.tile_pool(name="aux_psum", bufs=3, space="PSUM"))

            # ---------------- consolidated input loads ----------------
            # scatter-path inputs first: they gate the serial E-build chain
            misc = pool.tile([P, MISC_W], f32, tag="misc")
            nc.sync.dma_start(misc[:], misc_d[:])
            sidx_sb = pool.tile([P, F], i16, tag="sidx")
            nc.sync.dma_start(sidx_sb[:], sidx_d[:])
            svals_sb = pool.tile([P, M * F], f32, tag="svals")
            nc.sync.dma_start(svals_sb[:], svals_d[:])
            big0 = pool.tile([P, BIG0_W], f32, tag="big0")
            nc.sync.dma_start(big0[:], big0_d[:])

            ident = misc[:, _MO_ID:_MO_ID + 128]
            b1_ap = misc[:, _MO_B1:_MO_B1 + 16]
            b2_ap = misc[:, _MO_B2:_MO_B2 + 64]
            lb_ap = misc[:, _MO_LB:_MO_LB + 16]
            gw1_ap = misc[0:64, _MO_GW1:_MO_GW1 + 16]
            gw2_ap = misc[0:16, _MO_GW2:_MO_GW2 + 64]
            lw_ap = misc[0:64, _MO_LW:_MO_LW + 16]
            tidx_ap = misc[:, _MO_TIDX:_MO_TIDX + 4].bitcast(i32)

            prev_y = None
            stage_state = {}

            def _stage(name, tile_ref):
                stage_state["last"] = tile_ref
                if stop_after == name:
                    raise _StageStop()

            for rep in range(reps):
                try:
                    # ---------- filt = softmax(conv_w) / scale ----------
                    ex = pool.tile([P, L * C * T], f32, tag="ex")
                    nc.scalar.activation(ex[:],
                                         misc[:, _MO_CONV:_MO_CONV + 12],
                                         AF.Exp)
                    sums = pool.tile([P, L * C], f32, tag="sums")
                    nc.vector.tensor_reduce(
                        sums[:], ex[:].rearrange("p (g t) -> p g t", t=T),
                        axis=mybir.AxisListType.X, op=OP.add)
                    rec = pool.tile([P, L * C], f32, tag="rec")
                    nc.vector.reciprocal(rec[:], sums[:])
                    filt = pool.tile([P, L * C * T], f32, tag="filt")
                    for g in range(L * C):
                        nc.vector.tensor_scalar_mul(
                            filt[:, g * T:(g + 1) * T],
                            ex[:, g * T:(g + 1) * T], rec[:, g:g + 1])
                    filt_s = pool.tile([P, L * C * T], f32, tag="filt_s")
                    nc.scalar.activation(filt_s[:], filt[:], AF.Copy, bias=0.0,
                                         scale=1.0 / scale)

                    def fs(l, c, t):
                        q = (l * C + c) * T + t
                        return filt_s[:, q:q + 1]

                    # ---------- E build: quantize + scatter ----------
                    # dup cells sit in slots [0, DUPW) of each chunk: sum the
                    # m>=1 value planes into plane 0 over that window only.
                    svv = svals_sb[:].rearrange("p (m c s) -> p m c s",
                                                m=M, c=NCH)
                    vsum = svv[:, 0, :, :].rearrange("p c s -> p (c s)")
                    if M > 1 and DUPW > 0 and rep == 0:
                        d0 = svv[:, 0, :, 0:DUPW]
                        for m in range(1, M):
                            nc.vector.tensor_add(d0, d0,
                                                 svv[:, m, :, 0:DUPW])
                    if prev_y is not None:
                        jz = pool.tile([P, 1], f32, tag="jz")
                        nc.vector.tensor_scalar_mul(jz[:], prev_y, 0.0)
                        nc.vector.tensor_scalar_add(vsum[:, 0:1],
                                                    vsum[:, 0:1], jz[:, :])
                    vq_f = pool.tile([P, F], f32, tag="vq_f")
                    nc.scalar.activation(vq_f[:], vsum, AF.Copy, bias=0.0,
                                         scale=scale)
                    vq = pool.tile([P, F], i16, tag="vq")
                    nc.vector.tensor_copy(vq[:], vq_f[:])

                    eqh = [pool.tile([P, T * 2048], i16, tag=f"eq{hh}",
                                     name=f"eq{hh}") for hh in range(2)]
                    # scatter in q-major order so mats halves start early
                    for q in range(NCH // T):
                        for t in range(T):
                            ch = t * (NCH // T) + q
                            dst = eqh[q // 2][:, t * 2048 + (q % 2) * NEL:
                                              t * 2048 + (q % 2) * NEL + NEL]
                            nc.gpsimd.local_scatter(
                                out_ap=dst,
                                data_ap=vq[:, ch * NI:(ch + 1) * NI],
                                idxs_ap=sidx_sb[:, ch * NI:(ch + 1) * NI],
                                channels=P, num_elems=NEL, num_idxs=NI)

                    # ------- mats_l[c] = sum_t filt_s[l,c,t] * E_t -------
                    # mats0 now (gates pass A); mats1 is emitted after pass
                    # A so it overlaps pass A + the first AllGather. Each
                    # (l, c) is a pair of half tiles [P, 2048] (kc 0-7 /
                    # 8-15) so the build pipelines with the scatters.
                    mats = [[[pool.tile([P, MFREE // 2], f32,
                                        tag=f"mats{l}{c}{hh}",
                                        name=f"mats{l}{c}{hh}")
                              for hh in range(2)]
                             for c in range(C)] for l in range(L)]

                    def build_mats(l):
                        for hh in range(2):
                            for c in range(C):
                                dst = mats[l][c][hh][:]
                                nc.vector.tensor_scalar_mul(
                                    dst, eqh[hh][:, 0:2048], fs(l, c, 0))
                                for t in range(1, T):
                                    nc.vector.scalar_tensor_tensor(
                                        out=dst,
                                        in0=eqh[hh][:, t * 2048:
                                                    (t + 1) * 2048],
                                        scalar=fs(l, c, t), in1=dst,
                                        op0=OP.mult, op1=OP.add)

                    build_mats(0)
                    _stage("ebuild", mats[0][0][0][:, 0:1])

                    def mchunk(l, c, kc, mb):
                        # kc 0-7 in half 0 (q 0,1), kc 8-15 in half 1.
                        # within half: free = (kc % 8) * NS + r
                        o = (kc % 8) * NS + mb * P
                        return mats[l][c][kc // 8][:, o:o + P]

                    # ---------- X_ = X @ Ws -> rhs_a [X0|1|X1|1] ----------
                    rhs_a = pool.tile([P, KC, 130], f32, tag="rhs_a")
                    nc.vector.memset(rhs_a[:], 1.0)
                    for kc in range(KC):
                        ps = apool.tile([P, C * W_OUT], f32, space="PSUM",
                                        tag="aux")
                        for a in range(2):
                            nc.tensor.matmul(
                                ps[:],
                                big0[:, XT_OFF + a * N + kc * P:
                                     XT_OFF + a * N + (kc + 1) * P],
                                big0[:, WS_OFF + a * C * W_OUT:
                                     WS_OFF + (a + 1) * C * W_OUT],
                                start=(a == 0), stop=(a == 1))
                        nc.vector.tensor_copy(
                            rhs_a[:, kc, :].rearrange(
                                "p (b q) -> p b q", q=65)[:, :, 0:64],
                            ps[:].rearrange("p (b q) -> p b q", q=64))
                    xmy_sb = pool.tile([P, 2, C * W_OUT], f32, tag="xmy")
                    for mb in range(2):
                        ps = apool.tile([P, C * W_OUT], f32, space="PSUM",
                                        tag="aux")
                        for a in range(2):
                            nc.tensor.matmul(
                                ps[:],
                                big0[:, XMY_OFF + a * NS + mb * P:
                                     XMY_OFF + a * NS + (mb + 1) * P],
                                big0[:, WS_OFF + a * C * W_OUT:
                                     WS_OFF + (a + 1) * C * W_OUT],
                                start=(a == 0), stop=(a == 1))
                        nc.vector.tensor_copy(xmy_sb[:, mb, :], ps[:])

                    def allgather(name, shard_sb):
                        cci, cco, d = ccds[rep][name]
                        cciv = cci[:].rearrange("(mb p) d -> mb p d", p=P)
                        for mb in range(2):
                            nc.sync.dma_start(cciv[mb], shard_sb[:, mb, :])
                        nc.gpsimd.collective_compute(
                            "AllGather", OP.bypass, replica_groups=rg,
                            ins=[cci[:]], outs=[cco[:]])
                        rhs = pool.tile([P, KC, d], f32, tag=f"rhs_{name}")
                        nc.sync.dma_start(
                            rhs[:],
                            cco[:].rearrange("(p k) d -> p k d", p=P))
                        return rhs

                    # ================ PASS A (GT layer 0) ================
                    shA = pool.tile([P, 2, 130], f32, tag="shA")
                    for c in range(C):
                        for mb in range(2):
                            ps = ppool.tile([P, 65], f32, space="PSUM",
                                            tag="ep")
                            for kc in range(KC):
                                nc.tensor.matmul(
                                    ps[:], mchunk(0, c, kc, mb),
                                    rhs_a[:, kc, 65 * c:65 * c + 65],
                                    start=(kc == 0), stop=(kc == KC - 1))
                            nc.vector.tensor_copy(
                                shA[:, mb, 65 * c:65 * c + 65], ps[:])
                    build_mats(1)   # overlaps pass A + AllGather A
                    rhs_b = allgather("A", shA)
                    _stage("passA", rhs_b[:, 0, 0:1])

                    # ================ PASS B (GT layer 1) ================
                    psB = [[None] * 2 for _ in range(C)]
                    for c in range(C):
                        for mb in range(2):
                            ps = ppool.tile([P, 65], f32, space="PSUM",
                                            tag="ep")
                            psB[c][mb] = ps
                            for kc in range(KC):
                                nc.tensor.matmul(
                                    ps[:], mchunk(1, c, kc, mb),
                                    rhs_b[:, kc, 65 * c:65 * c + 65],
                                    start=(kc == 0), stop=(kc == KC - 1))
                    dinv = pool.tile([P, 2, 1], f32, tag="dinv")
                    hc_sb = pool.tile([P, 2, W_OUT], f32, tag="hc")
                    hcT_sb = pool.tile([W_OUT, NS], f32, tag="hcT")
                    w1_sb = pool.tile([P, 2, 16], f32, tag="w1")
                    for mb in range(2):
                        dg = pool.tile([P, 1], f32, tag="deg", bufs=2)
                        nc.vector.tensor_scalar_add(dg[:],
                                                    psB[0][mb][:, 64:65], 1.0)
                        nc.vector.tensor_add(dg[:], dg[:],
                                             psB[1][mb][:, 64:65])
                        sq = pool.tile([P, 1], f32, tag="sq", bufs=2)
                        nc.scalar.activation(sq[:], dg[:], AF.Sqrt)
                        nc.vector.reciprocal(dinv[:, mb, :], sq[:])
                        rsum = pool.tile([P, W_OUT], f32, tag="rsum", bufs=2)
                        for c in range(C):
                            tmp = pool.tile([P, W_OUT], f32, tag="hctmp",
                                            bufs=2)
                            nc.vector.tensor_add(
                                tmp[:], xmy_sb[:, mb, 64 * c:64 * c + 64],
                                psB[c][mb][:, 0:64])
                            if c == 0:
                                nc.scalar.activation(rsum[:], tmp[:],
                                                     AF.Relu, scale=BETA)
                            else:
                                r2 = pool.tile([P, W_OUT], f32,
                                               tag="hctmp2", bufs=2)
                                nc.scalar.activation(r2[:], tmp[:], AF.Relu,
                                                     scale=BETA)
                                nc.vector.tensor_add(rsum[:], rsum[:], r2[:])
                        nc.vector.tensor_scalar_mul(hc_sb[:, mb, :], rsum[:],
                                                    0.5)
                        tp = apool.tile([P, P], f32, space="PSUM", tag="aux")
                        nc.tensor.transpose(tp[:W_OUT, :], hc_sb[:, mb, :],
                                            ident)
                        nc.vector.tensor_copy(
                            hcT_sb[:, mb * P:(mb + 1) * P], tp[:W_OUT, :])
                    for mb in range(2):
                        psz = apool.tile([P, 16], f32, space="PSUM",
                                         tag="aux")
                        nc.tensor.matmul(psz[:],
                                         hcT_sb[:, mb * P:(mb + 1) * P],
                                         gw1_ap, start=True, stop=True)
                        nc.vector.tensor_scalar_mul(w1_sb[:, mb, :], psz[:],
                                                    dinv[:, mb, :])
                    rhs_c = allgather("C", w1_sb)
                    _stage("passB", rhs_c[:, 0, 0:1])

                    # ================ PASS C (GCN1 mats0) ================
                    shC = pool.tile([P, 2, C * 16], f32, tag="shC")
                    for c in range(C):
                        for mb in range(2):
                            ps = ppool.tile([P, 16], f32, space="PSUM",
                                            tag="ep")
                            for kc in range(KC):
                                nc.tensor.matmul(
                                    ps[:], mchunk(0, c, kc, mb),
                                    rhs_c[:, kc, :],
                                    start=(kc == 0), stop=(kc == KC - 1))
                            nc.vector.tensor_copy(
                                shC[:, mb, 16 * c:16 * c + 16], ps[:])
                    rhs_d = allgather("D", shC)
                    _stage("passC", rhs_d[:, 0, 0:1])

                    # ================ PASS D (GCN1 mats1) ================
                    h_sb = pool.tile([P, 2, 16], f32, tag="h")
                    hT_sb = pool.tile([16, NS], f32, tag="hT")
                    w2_sb = pool.tile([P, 2, W_OUT], f32, tag="w2")
                    for mb in range(2):
                        ps = ppool.tile([P, 16], f32, space="PSUM", tag="ep")
                        first = True
                        for c in range(C):
                            for kc in range(KC):
                                nc.tensor.matmul(
                                    ps[:], mchunk(1, c, kc, mb),
                                    rhs_d[:, kc, 16 * c:16 * c + 16],
                                    start=first,
                                    stop=(c == C - 1 and kc == KC - 1))
                                first = False
                        aw = pool.tile([P, 16], f32, tag="aw1", bufs=2)
                        nc.vector.tensor_add(aw[:], ps[:], w1_sb[:, mb, :])
                        nc.vector.tensor_scalar_mul(aw[:], aw[:],
                                                    dinv[:, mb, :])
                        nc.vector.tensor_add(aw[:], aw[:], b1_ap)
                        nc.vector.tensor_scalar_max(h_sb[:, mb, :], aw[:],
                                                    0.0)
                        tp = apool.tile([P, P], f32, space="PSUM", tag="aux")
                        nc.tensor.transpose(tp[:16, :], h_sb[:, mb, :],
                                            ident)
                        nc.vector.tensor_copy(
                            hT_sb[:, mb * P:(mb + 1) * P], tp[:16, :])
                    for mb in range(2):
                        psz = apool.tile([P, W_OUT], f32, space="PSUM",
                                         tag="aux")
                        nc.tensor.matmul(psz[:],
                                         hT_sb[:, mb * P:(mb + 1) * P],
                                         gw2_ap, start=True, stop=True)
                        nc.vector.tensor_scalar_mul(w2_sb[:, mb, :], psz[:],
                                                    dinv[:, mb, :])
                    rhs_e = allgather("E", w2_sb)
                    _stage("passD", rhs_e[:, 0, 0:1])

                    # ================ PASS E (GCN2 mats0) ================
                    shE = pool.tile([P, 2, C * W_OUT], f32, tag="shE")
                    for c in range(C):
                        for mb in range(2):
                            ps = ppool.tile([P, W_OUT], f32, space="PSUM",
                                            tag="ep")
                            for kc in range(KC):
                                nc.tensor.matmul(
                                    ps[:], mchunk(0, c, kc, mb),
                                    rhs_e[:, kc, :],
                                    start=(kc == 0), stop=(kc == KC - 1))
                            nc.vector.tensor_copy(
                                shE[:, mb, 64 * c:64 * c + 64], ps[:])
                    rhs_f = allgather("F", shE)
                    _stage("passE", rhs_f[:, 0, 0:1])

                    # ========== PASS F (GCN2 mats1) + log_softmax ==========
                    hls_sb = pool.tile([P, 2, W_OUT], f32, tag="hls")
                    for mb in range(2):
                        ps = ppool.tile([P, W_OUT], f32, space="PSUM",
                                        tag="ep")
                        first = True
                        for c in range(C):
                            for kc in range(KC):
                                nc.tensor.matmul(
                                    ps[:], mchunk(1, c, kc, mb),
                                    rhs_f[:, kc, 64 * c:64 * c + 64],
                                    start=first,
                                    stop=(c == C - 1 and kc == KC - 1))
                                first = False
                        aw = pool.tile([P, W_OUT], f32, tag="aw2", bufs=2)
                        nc.vector.tensor_add(aw[:], ps[:], w2_sb[:, mb, :])
                        nc.vector.tensor_scalar_mul(aw[:], aw[:],
                                                    dinv[:, mb, :])
                        nc.vector.tensor_add(aw[:], aw[:], b2_ap)
                        mx = pool.tile([P, 1], f32, tag="mx", bufs=2)
                        nc.vector.tensor_reduce(mx[:], aw[:],
                                                axis=mybir.AxisListType.X,
                                                op=OP.max)
                        nmx = pool.tile([P, 1], f32, tag="nmx", bufs=2)
                        nc.vector.tensor_scalar_mul(nmx[:], mx[:], -1.0)
                        ee = pool.tile([P, W_OUT], f32, tag="ee", bufs=2)
                        nc.scalar.activation(ee[:], aw[:], AF.Exp,
                                             bias=nmx[:, :])
                        ssum = pool.tile([P, 1], f32, tag="ssum", bufs=2)
                        nc.vector.tensor_reduce(ssum[:], ee[:],
                                                axis=mybir.AxisListType.X,
                                                op=OP.add)
                        lns = pool.tile([P, 1], f32, tag="lns", bufs=2)
                        nc.scalar.activation(lns[:], ssum[:], AF.Ln)
                        tot = pool.tile([P, 1], f32, tag="tot", bufs=2)
                        nc.vector.tensor_add(tot[:], mx[:], lns[:])
                        nc.vector.tensor_scalar(out=hls_sb[:, mb, :],
                                                in0=aw[:], scalar1=tot[:, :],
                                                scalar2=None,
                                                op0=OP.subtract)

                    # -------- AG h, gather targets, linear head --------
                    cci, cco, _ = ccds[rep]["H"]
                    cciv = cci[:].rearrange("(mb p) d -> mb p d", p=P)
                    for mb in range(2):
                        nc.sync.dma_start(cciv[mb], hls_sb[:, mb, :])
                    nc.gpsimd.collective_compute(
                        "AllGather", OP.bypass, replica_groups=rg,
                        ins=[cci[:]], outs=[cco[:]])
                    hloc = nc.dram_tensor(f"hloc{rep}", [N, W_OUT], f32)
                    nc.sync.dma_start(hloc[:], cco[:])
                    gt = pool.tile([P, NTGT // P, W_OUT], f32, tag="gt")
                    for b in range(NTGT // P):
                        nc.gpsimd.indirect_dma_start(
                            out=gt[:, b, :], out_offset=None, in_=hloc[:],
                            in_offset=bass.IndirectOffsetOnAxis(
                                ap=tidx_ap[:, b:b + 1], axis=0))
                    gT_sb = pool.tile([W_OUT, NTGT], f32, tag="gT")
                    for b in range(NTGT // P):
                        tp = apool.tile([P, P], f32, space="PSUM", tag="aux")
                        nc.tensor.transpose(tp[:W_OUT, :], gt[:, b, :],
                                            ident)
                        nc.vector.tensor_copy(
                            gT_sb[:, b * P:(b + 1) * P], tp[:W_OUT, :])
                    y_sb = pool.tile([P, NTGT // P, NCLS], f32, tag="y_sb")
                    for b in range(NTGT // P):
                        psy = apool.tile([P, NCLS], f32, space="PSUM",
                                         tag="aux")
                        nc.tensor.matmul(psy[:],
                                         gT_sb[:, b * P:(b + 1) * P],
                                         lw_ap, start=True, stop=True)
                        nc.vector.tensor_add(y_sb[:, b, :], psy[:], lb_ap)
                    yv = y_d[:].rearrange("(b p) n -> b p n", p=P)
                    for b in range(NTGT // P):
                        nc.sync.dma_start(yv[b], y_sb[:, b, :])
                except _StageStop:
                    lt = stage_state["last"]
                    y_sb = pool.tile([P, NTGT // P, NCLS], f32, tag="ydummy")
                    nc.vector.memset(y_sb[:], 0.0)
                    nc.vector.tensor_scalar_mul(y_sb[:, 0, 0:1], lt, 0.0)
                    yv = y_d[:].rearrange("(b p) n -> b p n", p=P)
                    for b in range(NTGT // P):
                        nc.sync.dma_start(yv[b], y_sb[:, b, :])
                prev_y = y_sb[:, 0, 0:1]

    nc.compile()
    return nc


# ---------------------------------------------------------------------------
# Execution via PJRT (axon) with a persistent jitted callable.
# ---------------------------------------------------------------------------
class _Runner:
    def __init__(self, nc, n_cores):
        import jax
        from jax.sharding import Mesh, PartitionSpec
        from jax.experimental.shard_map import shard_map
        from concourse.bass2jax import (
            _bass_exec_p, install_neuronx_cc_hook, partition_id_tensor)

        install_neuronx_cc_hook()
        self.jax = jax
        self._nc = nc
        self.n_cores = n_cores
        partition_name = (
            nc.partition_id_tensor.name if nc.partition_id_tensor else None)
        in_names, out_names, out_avals, zero_outs = [], [], [], []
        for alloc in nc.m.functions[0].allocations:
            if not isinstance(alloc, mybir.MemoryLocationSet):
                continue
            name = alloc.memorylocations[0].name
            if alloc.kind == "ExternalInput":
                if name != partition_name:
                    in_names.append(name)
            elif alloc.kind == "ExternalOutput":
                shape = tuple(alloc.tensor_shape)
                dtype = mybir.dt.np(alloc.dtype)
                out_names.append(name)
                out_avals.append(jax.core.ShapedArray(shape, dtype))
                zero_outs.append(np.zeros(shape, dtype))
        self.n_params = len(in_names)
        self.out_names = out_names
        self.out_avals = out_avals
        self.zero_outs = zero_outs
        n_outs = len(out_avals)
        in_names = in_names + out_names
        if partition_name is not None:
            in_names.append(partition_name)
        self.in_names = in_names

        def _body(*args):
            operands = list(args)
            if partition_name is not None:
                operands.append(partition_id_tensor())
            outs = _bass_exec_p.bind(
                *operands, out_avals=tuple(out_avals),
                in_names=tuple(in_names), out_names=tuple(out_names),
                lowering_input_output_aliases=(),
                sim_require_finite=True, sim_require_nnan=True, nc=nc)
            return tuple(outs)

        devices = jax.devices()[:n_cores]
        mesh = Mesh(np.asarray(devices), ("core",))
        in_specs = (PartitionSpec("core"),) * (self.n_params + n_outs)
        out_specs = (PartitionSpec("core"),) * n_outs
        self._fn = jax.jit(
            shard_map(_body, mesh=mesh, in_specs=in_specs,
                      out_specs=out_specs, check_rep=False),
            donate_argnums=tuple(range(self.n_params,
                                       self.n_params + n_outs)),
            keep_unused=True)

    def concat_inputs(self, in_maps):
        return [
            np.concatenate([np.asarray(m[name]) for m in in_maps], axis=0)
            for name in self.in_names[: self.n_params]
        ]

    def zeros(self):
        return [
            np.zeros((self.n_cores * z.shape[0], *z.shape[1:]), z.dtype)
            for z in self.zero_outs
        ]

    def run(self, in_maps):
        outs = self._fn(*self.concat_inputs(in_maps), *self.zeros())
        return [
            {
                name: np.asarray(outs[i]).reshape(
                    self.n_cores, *self.out_avals[i].shape)[c]
                for i, name in enumerate(self.out_names)
            }
            for c in range(self.n_cores)
        ]


_CACHE = {}


def _get_runner(NI, M, scale, DUPW=0, reps=1, stop_after=None):
    key = (NI, M, scale, DUPW, reps, stop_after)
    if key not in _CACHE:
        nc = build_kernel(NI, M, scale, DUPW=DUPW, reps=reps,
                          stop_after=stop_after)
        _CACHE[key] = _Runner(nc, NCORES)
    return _CACHE[key]


def kernel(**inputs) -> np.ndarray:
    in_maps, NI, M, scale, DUPW = _prep_inputs(**inputs)
    runner = _get_runner(NI, M, scale, DUPW)
    results = runner.run(in_maps)
    return results[0]["y"]



# revision 2
# speedup vs baseline: 11.9827x; 11.9827x over previous
"""Trainium2 Bass kernel v2 for nn_EncodingNet (FastGTN-style GNN).

Changes vs v1: all GEMM operands/collective payloads in bf16 (PE 4x, DVE 2x,
half-size AllGathers), and the final AllGather+head is replaced by a local
head: each core computes y rows for the target nodes it owns (one-hot select
matmul + indirect-DMA scatter into its y buffer); the host sums the 8 per-core
y buffers (rows are disjoint).
"""

import os
import sys
import types

import numpy as np

if "antenv.axon_hooks" not in sys.modules:
    _m = types.ModuleType("antenv.axon_hooks")
    _m.get_axon_ntff_profile_hook = lambda: None
    sys.modules["antenv.axon_hooks"] = _m

import ml_dtypes

import concourse.bass as bass
import concourse.bacc as bacc
import concourse.tile as tile
from concourse import mybir

N = 2048
C = 2
T = 3
L = 2
E = 65536
W_IN = 256
W_OUT = 64
NCLS = 16
NTGT = 512
BETA = 0.5

NCORES = 8
NS = N // NCORES
P = 128
KC = 16
NEL = 1024
NCH = (T * KC * NS) // NEL
EFREE = T * KC * NS
MFREE = KC * NS

f32 = mybir.dt.float32
bf16 = mybir.dt.bfloat16
i16 = mybir.dt.int16
i32 = mybir.dt.int32
AF = mybir.ActivationFunctionType
OP = mybir.AluOpType

# misc pack offsets (columns in the [128, MISC_W] fp32 misc tensor)
_MO_CONV = 0          # [12]
_MO_B1 = 12           # [16]
_MO_B2 = 28           # [64]
_MO_LB = 92           # [16]
_MO_YIDX = 108        # [1] int32 bits
MISC_W = 112

# aux pack offsets (columns in the [128, AUX_W] bf16 aux tensor)
_AO_ID = 0            # [128] identity
_AO_GW1 = 128         # [16]  (partitions 0..63)
_AO_GW2 = 144         # [64]  (partitions 0..15)
_AO_LW = 208          # [16]  (partitions 0..63)
_AO_CSN = 224         # [16]  -colsum(lin_w), replicated
_AO_SEL0 = 240        # [128] one-hot select, local rows 0..127
_AO_SEL1 = 368        # [128] one-hot select, local rows 128..255
AUX_W = 496


def _prep_inputs(X, edge_value, conv_w, Ws, gcn_w1, gcn_b1, gcn_w2, gcn_b2,
                 lin_w, lin_b, edge_index, target_x):
    X = np.asarray(X, np.float32)
    edge_value = np.asarray(edge_value, np.float32)
    conv_w = np.asarray(conv_w, np.float32)
    Ws = np.asarray(Ws, np.float32)
    gcn_w1 = np.asarray(gcn_w1, np.float32)
    gcn_b1 = np.asarray(gcn_b1, np.float32)
    gcn_w2 = np.asarray(gcn_w2, np.float32)
    gcn_b2 = np.asarray(gcn_b2, np.float32)
    lin_w = np.asarray(lin_w, np.float32)
    lin_b = np.asarray(lin_b, np.float32)
    ei = np.asarray(edge_index, np.int64)
    tx = np.asarray(target_x, np.int64)

    kk, pp = np.meshgrid(np.arange(KC), np.arange(P), indexing="ij")
    pos_node = (pp * KC + kk).reshape(-1)
    xT_perm = np.ascontiguousarray(X[pos_node].T)          # [256, 2048]
    ws_cat = np.concatenate([Ws[0], Ws[1]], axis=1)        # [256, 128]

    def fold(a):  # [256, F] -> [128, 2*F]
        fdim = a.shape[1]
        return np.ascontiguousarray(
            a.reshape(2, P, fdim).transpose(1, 0, 2).reshape(P, 2 * fdim))

    misc = np.zeros((P, MISC_W), np.float32)
    misc[:, _MO_CONV:_MO_CONV + 12] = conv_w.reshape(1, -1)
    misc[:, _MO_B1:_MO_B1 + 16] = gcn_b1.reshape(1, -1)
    misc[:, _MO_B2:_MO_B2 + 64] = gcn_b2.reshape(1, -1)
    misc[:, _MO_LB:_MO_LB + 16] = lin_b.reshape(1, -1)

    aux = np.zeros((P, AUX_W), np.float32)
    aux[:, _AO_ID:_AO_ID + 128] = np.eye(P, dtype=np.float32)
    aux[:64, _AO_GW1:_AO_GW1 + 16] = gcn_w1
    aux[:16, _AO_GW2:_AO_GW2 + 64] = gcn_w2
    aux[:64, _AO_LW:_AO_LW + 16] = lin_w
    aux[:, _AO_CSN:_AO_CSN + 16] = -lin_w.sum(axis=0).reshape(1, -1)

    # ---- edge bucketing per core (index-only) ----
    t_id = np.repeat(np.arange(T, dtype=np.int64), E)
    r_all = ei[:, 0, :].reshape(-1)
    c_all = ei[:, 1, :].reshape(-1)
    v_all = edge_value.reshape(-1)
    rank = r_all >> 8
    r_loc = r_all & 255
    p_of = c_all >> 4
    kc_of = c_all & 15
    free = t_id * MFREE + kc_of * NS + r_loc
    ch_of = free // NEL
    pos_of = free % NEL
    bucket = (rank * P + p_of) * NCH + ch_of
    cell = bucket * NEL + pos_of

    order = np.argsort(cell, kind="stable")
    cell_s = cell[order]
    v_s = v_all[order]
    ucell, first_idx, counts = np.unique(cell_s, return_index=True,
                                         return_counts=True)
    occ = np.arange(len(cell_s)) - np.repeat(first_idx, counts)
    M = int(counts.max())
    ubucket = ucell // NEL
    upos = ucell % NEL
    order2 = np.lexsort((np.arange(len(ucell)), counts == 1, ubucket))
    inv2 = np.empty_like(order2)
    inv2[order2] = np.arange(len(order2))
    ub_sorted = ubucket[order2]
    ub_uniq, ub_fidx, ub_counts = np.unique(ub_sorted, return_index=True,
                                            return_counts=True)
    slot_sorted = np.arange(len(ucell)) - np.repeat(ub_fidx, ub_counts)
    slot = slot_sorted[inv2]
    ndup_per_bucket = np.zeros(len(ub_uniq), np.int64)
    isdup_sorted = (counts[order2] >= 2)
    np.add.at(ndup_per_bucket,
              np.searchsorted(ub_uniq, ub_sorted), isdup_sorted)
    DUPW = int(ndup_per_bucket.max()) if M > 1 else 0
    max_cnt = int(ub_counts.max())
    NI = max_cnt + (max_cnt & 1)
    scale = float(2.0 ** np.floor(np.log2(32767.0 / M)))

    scat_idx = np.full((NCORES, P, NCH * NI), -1, np.int16)
    scat_vals = np.zeros((NCORES, M, P, NCH * NI), np.float32)
    uk = ubucket // (P * NCH)
    up = (ubucket // NCH) % P
    uch = ubucket % NCH
    scat_idx[uk, up, uch * NI + slot] = upos.astype(np.int16)
    cell_row = np.searchsorted(ucell, cell_s)
    scat_vals[uk[cell_row], occ, up[cell_row],
              uch[cell_row] * NI + slot[cell_row]] = v_s

    big0_shared = np.concatenate([fold(xT_perm), fold(ws_cat)], axis=1)
    in_maps = []
    for k in range(NCORES):
        # per-core target slots (row ownership)
        tloc = [(i, int(tx[i]) - k * NS) for i in range(NTGT)
                if k * NS <= tx[i] < (k + 1) * NS]
        assert 1 <= len(tloc) <= P, f"core {k}: {len(tloc)} targets"
        tloc = tloc + [tloc[0]] * (P - len(tloc))
        auxk = aux.copy()
        yidx = np.zeros(P, np.int32)
        for s, (i, r) in enumerate(tloc):
            yidx[s] = i
            if r < 128:
                auxk[r, _AO_SEL0 + s] = 1.0
            else:
                auxk[r - 128, _AO_SEL1 + s] = 1.0
        misck = misc.copy()
        misck[:, _MO_YIDX] = yidx.view(np.float32)

        xmy = fold(np.ascontiguousarray(X[k * NS:(k + 1) * NS].T))
        m = {
            "big0": np.ascontiguousarray(np.concatenate(
                [big0_shared, xmy], axis=1)).astype(ml_dtypes.bfloat16),
            "misc": misck,
            "aux": auxk.astype(ml_dtypes.bfloat16),
            "sidx": scat_idx[k],
            "svals": np.ascontiguousarray(
                scat_vals[k].transpose(1, 0, 2).reshape(P, -1)),
        }
        in_maps.append(m)
    return in_maps, NI, M, scale, DUPW


class _StageStop(Exception):
    pass


def build_kernel(NI, M, scale, DUPW=0, reps=1, stop_after=None):
    nc = bacc.Bacc("TRN2", target_bir_lowering=False, debug=False,
                   num_devices=NCORES)
    F = NCH * NI
    XT_OFF = 0
    WS_OFF = 2 * N
    XMY_OFF = 2 * N + 2 * C * W_OUT
    BIG0_W = XMY_OFF + 2 * NS

    big0_d = nc.dram_tensor("big0", [P, BIG0_W], bf16, kind="ExternalInput")
    misc_d = nc.dram_tensor("misc", [P, MISC_W], f32, kind="ExternalInput")
    aux_d = nc.dram_tensor("aux", [P, AUX_W], bf16, kind="ExternalInput")
    sidx_d = nc.dram_tensor("sidx", [P, F], i16, kind="ExternalInput")
    svals_d = nc.dram_tensor("svals", [P, M * F], f32, kind="ExternalInput")
    y_d = nc.dram_tensor("y", [NTGT, NCLS], f32, kind="ExternalOutput")

    ccds = []
    for r in range(reps):
        ccd = {}
        for name, d in [("A", 130), ("C", 16), ("D", C * 16),
                        ("E", W_OUT), ("F", C * W_OUT)]:
            ccd[name] = (
                nc.dram_tensor(f"cci_{name}{r}", [NS, d], bf16),
                nc.dram_tensor(f"cco_{name}{r}", [N, d], bf16,
                               addr_space="Shared"),
                d,
            )
        ccds.append(ccd)
    rg = [list(range(NCORES))]

    with tile.TileContext(nc) as tc:
        import contextlib
        ctx = contextlib.ExitStack()
        with ctx:
            pool = ctx.enter_context(tc.tile_pool(name="main", bufs=1))
            ppool = ctx.enter_context(
                tc.tile_pool(name="pass_psum", bufs=4, space="PSUM"))
            apool = ctx.enter_context(
                tc.tile_pool(name="aux_psum", bufs=2, space="PSUM"))

            misc = pool.tile([P, MISC_W], f32, tag="misc")
            nc.sync.dma_start(misc[:], misc_d[:])
            aux = pool.tile([P, AUX_W], bf16, tag="aux")
            nc.sync.dma_start(aux[:], aux_d[:])
            sidx_sb = pool.tile([P, F], i16, tag="sidx")
            nc.sync.dma_start(sidx_sb[:], sidx_d[:])
            svals_sb = pool.tile([P, M * F], f32, tag="svals")
            nc.sync.dma_start(svals_sb[:], svals_d[:])
            big0 = pool.tile([P, BIG0_W], bf16, tag="big0")
            nc.sync.dma_start(big0[:], big0_d[:])

            b1_ap = misc[:, _MO_B1:_MO_B1 + 16]
            b2_ap = misc[:, _MO_B2:_MO_B2 + 64]
            lb_ap = misc[:, _MO_LB:_MO_LB + 16]
            yidx_ap = misc[:, _MO_YIDX:_MO_YIDX + 1].bitcast(i32)
            identb = aux[:, _AO_ID:_AO_ID + 128]
            gw1_ap = aux[0:64, _AO_GW1:_AO_GW1 + 16]
            gw2_ap = aux[0:16, _AO_GW2:_AO_GW2 + 64]
            lw_ap = aux[0:64, _AO_LW:_AO_LW + 16]
            csn_ap = aux[:, _AO_CSN:_AO_CSN + 16]
            sel0_ap = aux[:, _AO_SEL0:_AO_SEL0 + 128]
            sel1_ap = aux[:, _AO_SEL1:_AO_SEL1 + 128]

            prev_y = None
            stage_state = {}

            def _stage(name, tile_ref):
                stage_state["last"] = tile_ref
                if stop_after == name:
                    raise _StageStop()

            for rep in range(reps):
                try:
                    # ---------- filt = softmax(conv_w) / scale ----------
                    ex = pool.tile([P, L * C * T], f32, tag="ex")
                    nc.scalar.activation(ex[:],
                                         misc[:, _MO_CONV:_MO_CONV + 12],
                                         AF.Exp)
                    sums = pool.tile([P, L * C], f32, tag="sums")
                    nc.vector.tensor_reduce(
                        sums[:], ex[:].rearrange("p (g t) -> p g t", t=T),
                        axis=mybir.AxisListType.X, op=OP.add)
                    rec = pool.tile([P, L * C], f32, tag="rec")
                    nc.vector.reciprocal(rec[:], sums[:])
                    filt = pool.tile([P, L * C * T], f32, tag="filt")
                    for g in range(L * C):
                        nc.vector.tensor_scalar_mul(
                            filt[:, g * T:(g + 1) * T],
                            ex[:, g * T:(g + 1) * T], rec[:, g:g + 1])
                    filt_s = pool.tile([P, L * C * T], f32, tag="filt_s")
                    nc.scalar.activation(filt_s[:], filt[:], AF.Copy,
                                         bias=0.0, scale=1.0 / scale)

                    def fs(l, c, t):
                        q = (l * C + c) * T + t
                        return filt_s[:, q:q + 1]

                    # ---------- E build: quantize + scatter ----------
                    svv = svals_sb[:].rearrange("p (m c s) -> p m c s",
                                                m=M, c=NCH)
                    vsum = svv[:, 0, :, :].rearrange("p c s -> p (c s)")
                    if M > 1 and DUPW > 0 and rep == 0:
                        d0 = svv[:, 0, :, 0:DUPW]
                        for m in range(1, M):
                            nc.vector.tensor_add(d0, d0,
                                                 svv[:, m, :, 0:DUPW])
                    if prev_y is not None:
                        jz = pool.tile([P, 1], f32, tag="jz")
                        nc.vector.tensor_scalar_mul(jz[:], prev_y, 0.0)
                        nc.vector.tensor_scalar_add(vsum[:, 0:1],
                                                    vsum[:, 0:1], jz[:, :])
                    vq_f = pool.tile([P, F], f32, tag="vq_f")
                    nc.scalar.activation(vq_f[:], vsum, AF.Copy, bias=0.0,
                                         scale=scale)
                    vq = pool.tile([P, F], i16, tag="vq")
                    nc.vector.tensor_copy(vq[:], vq_f[:])

                    eqh = [pool.tile([P, T * 2048], i16, tag=f"eq{hh}",
                                     name=f"eq{hh}") for hh in range(2)]
                    for q in range(NCH // T):
                        for t in range(T):
                            ch = t * (NCH // T) + q
                            dst = eqh[q // 2][:, t * 2048 + (q % 2) * NEL:
                                              t * 2048 + (q % 2) * NEL + NEL]
                            nc.gpsimd.local_scatter(
                                out_ap=dst,
                                data_ap=vq[:, ch * NI:(ch + 1) * NI],
                                idxs_ap=sidx_sb[:, ch * NI:(ch + 1) * NI],
                                channels=P, num_elems=NEL, num_idxs=NI)

                    # ------- mats_l[c] = sum_t filt_s[l,c,t] * E_t -------
                    mats = [[[pool.tile([P, MFREE // 2], bf16,
                                        tag=f"mats{l}{c}{hh}",
                                        name=f"mats{l}{c}{hh}")
                              for hh in range(2)]
                             for c in range(C)] for l in range(L)]

                    def build_mats(l):
                        for hh in range(2):
                            for c in range(C):
                                dst = mats[l][c][hh][:]
                                nc.vector.tensor_scalar_mul(
                                    dst, eqh[hh][:, 0:2048], fs(l, c, 0))
                                for t in range(1, T):
                                    nc.vector.scalar_tensor_tensor(
                                        out=dst,
                                        in0=eqh[hh][:, t * 2048:
                                                    (t + 1) * 2048],
                                        scalar=fs(l, c, t), in1=dst,
                                        op0=OP.mult, op1=OP.add)

                    build_mats(0)
                    _stage("ebuild", mats[0][0][0][:, 0:1])

                    def mchunk(l, c, kc, mb):
                        o = (kc % 8) * NS + mb * P
                        return mats[l][c][kc // 8][:, o:o + P]

                    # ---------- X_ = X @ Ws -> rhs_a [X0|1|X1|1] ----------
                    rhs_a = pool.tile([P, KC, 130], bf16, tag="rhs_a")
                    nc.vector.memset(rhs_a[:], 1.0)
                    for kc in range(KC):
                        ps = apool.tile([P, C * W_OUT], f32, space="PSUM",
                                        tag="aux")
                        for a in range(2):
                            nc.tensor.matmul(
                                ps[:],
                                big0[:, XT_OFF + a * N + kc * P:
                                     XT_OFF + a * N + (kc + 1) * P],
                                big0[:, WS_OFF + a * C * W_OUT:
                                     WS_OFF + (a + 1) * C * W_OUT],
                                start=(a == 0), stop=(a == 1))
                        nc.vector.tensor_copy(
                            rhs_a[:, kc, :].rearrange(
                                "p (b q) -> p b q", q=65)[:, :, 0:64],
                            ps[:].rearrange("p (b q) -> p b q", q=64))
                    xmy_sb = pool.tile([P, 2, C * W_OUT], bf16, tag="xmy")
                    for mb in range(2):
                        ps = apool.tile([P, C * W_OUT], f32, space="PSUM",
                                        tag="aux")
                        for a in range(2):
                            nc.tensor.matmul(
                                ps[:],
                                big0[:, XMY_OFF + a * NS + mb * P:
                                     XMY_OFF + a * NS + (mb + 1) * P],
                                big0[:, WS_OFF + a * C * W_OUT:
                                     WS_OFF + (a + 1) * C * W_OUT],
                                start=(a == 0), stop=(a == 1))
                        nc.vector.tensor_copy(xmy_sb[:, mb, :], ps[:])

                    def allgather(name, shard_sb):
                        cci, cco, d = ccds[rep][name]
                        cciv = cci[:].rearrange("(mb p) d -> mb p d", p=P)
                        for mb in range(2):
                            nc.sync.dma_start(cciv[mb], shard_sb[:, mb, :])
                        nc.gpsimd.collective_compute(
                            "AllGather", OP.bypass, replica_groups=rg,
                            ins=[cci[:]], outs=[cco[:]])
                        rhs = pool.tile([P, KC, d], bf16, tag=f"rhs_{name}")
                        nc.sync.dma_start(
                            rhs[:],
                            cco[:].rearrange("(p k) d -> p k d", p=P))
                        return rhs

                    # ================ PASS A (GT layer 0) ================
                    shA = pool.tile([P, 2, 130], bf16, tag="shA")
                    for c in range(C):
                        for mb in range(2):
                            ps = ppool.tile([P, 65], f32, space="PSUM",
                                            tag="ep")
                            for kc in range(KC):
                                nc.tensor.matmul(
                                    ps[:], mchunk(0, c, kc, mb),
                                    rhs_a[:, kc, 65 * c:65 * c + 65],
                                    start=(kc == 0), stop=(kc == KC - 1))
                            nc.vector.tensor_copy(
                                shA[:, mb, 65 * c:65 * c + 65], ps[:])
                    build_mats(1)
                    rhs_b = allgather("A", shA)
                    _stage("passA", rhs_b[:, 0, 0:1])

                    # ================ PASS B (GT layer 1) ================
                    psB = [[None] * 2 for _ in range(C)]
                    for c in range(C):
                        for mb in range(2):
                            ps = ppool.tile([P, 65], f32, space="PSUM",
                                            tag="ep")
                            psB[c][mb] = ps
                            for kc in range(KC):
                                nc.tensor.matmul(
                                    ps[:], mchunk(1, c, kc, mb),
                                    rhs_b[:, kc, 65 * c:65 * c + 65],
                                    start=(kc == 0), stop=(kc == KC - 1))
                    dinv = pool.tile([P, 2, 1], f32, tag="dinv")
                    hc_sb = pool.tile([P, 2, W_OUT], bf16, tag="hc")
                    hcT_sb = pool.tile([W_OUT, NS], bf16, tag="hcT")
                    w1_sb = pool.tile([P, 2, 16], bf16, tag="w1")
                    for mb in range(2):
                        dg = pool.tile([P, 1], f32, tag="deg", bufs=2)
                        nc.vector.tensor_scalar_add(dg[:],
                                                    psB[0][mb][:, 64:65],
                                                    1.0)
                        nc.vector.tensor_add(dg[:], dg[:],
                                             psB[1][mb][:, 64:65])
                        sq = pool.tile([P, 1], f32, tag="sq", bufs=2)
                        nc.scalar.activation(sq[:], dg[:], AF.Sqrt)
                        nc.vector.reciprocal(dinv[:, mb, :], sq[:])
                        rsum = pool.tile([P, W_OUT], f32, tag="rsum", bufs=2)
                        for c in range(C):
                            tmp = pool.tile([P, W_OUT], f32, tag="hctmp",
                                            bufs=2)
                            nc.vector.tensor_add(
                                tmp[:], xmy_sb[:, mb, 64 * c:64 * c + 64],
                                psB[c][mb][:, 0:64])
                            if c == 0:
                                nc.scalar.activation(rsum[:], tmp[:],
                                                     AF.Relu, scale=BETA)
                            else:
                                r2 = pool.tile([P, W_OUT], f32,
                                               tag="hctmp2", bufs=2)
                                nc.scalar.activation(r2[:], tmp[:], AF.Relu,
                                                     scale=BETA)
                                nc.vector.tensor_add(rsum[:], rsum[:],
                                                     r2[:])
                        nc.vector.tensor_scalar_mul(hc_sb[:, mb, :],
                                                    rsum[:], 0.5)
                        tp = apool.tile([P, P], bf16, space="PSUM",
                                        tag="auxb")
                        nc.tensor.transpose(tp[:W_OUT, :], hc_sb[:, mb, :],
                                            identb)
                        nc.vector.tensor_copy(
                            hcT_sb[:, mb * P:(mb + 1) * P], tp[:W_OUT, :])
                    for mb in range(2):
                        psz = apool.tile([P, 16], f32, space="PSUM",
                                         tag="aux")
                        nc.tensor.matmul(psz[:],
                                         hcT_sb[:, mb * P:(mb + 1) * P],
                                         gw1_ap, start=True, stop=True)
                        nc.vector.tensor_scalar_mul(w1_sb[:, mb, :], psz[:],
                                                    dinv[:, mb, :])
                    rhs_c = allgather("C", w1_sb)
                    _stage("passB", rhs_c[:, 0, 0:1])

                    # ================ PASS C (GCN1 mats0) ================
                    shC = pool.tile([P, 2, C * 16], bf16, tag="shC")
                    for c in range(C):
                        for mb in range(2):
                            ps = ppool.tile([P, 16], f32, space="PSUM",
                                            tag="ep")
                            for kc in range(KC):
                                nc.tensor.matmul(
                                    ps[:], mchunk(0, c, kc, mb),
                                    rhs_c[:, kc, :],
                                    start=(kc == 0), stop=(kc == KC - 1))
                            nc.vector.tensor_copy(
                                shC[:, mb, 16 * c:16 * c + 16], ps[:])
                    rhs_d = allgather("D", shC)
                    _stage("passC", rhs_d[:, 0, 0:1])

                    # ================ PASS D (GCN1 mats1) ================
                    h_sb = pool.tile([P, 2, 16], bf16, tag="h")
                    hT_sb = pool.tile([16, NS], bf16, tag="hT")
                    w2_sb = pool.tile([P, 2, W_OUT], bf16, tag="w2")
                    for mb in range(2):
                        ps = ppool.tile([P, 16], f32, space="PSUM", tag="ep")
                        first = True
                        for c in range(C):
                            for kc in range(KC):
                                nc.tensor.matmul(
                                    ps[:], mchunk(1, c, kc, mb),
                                    rhs_d[:, kc, 16 * c:16 * c + 16],
                                    start=first,
                                    stop=(c == C - 1 and kc == KC - 1))
                                first = False
                        aw = pool.tile([P, 16], f32, tag="aw1", bufs=2)
                        nc.vector.tensor_add(aw[:], ps[:], w1_sb[:, mb, :])
                        nc.vector.tensor_scalar_mul(aw[:], aw[:],
                                                    dinv[:, mb, :])
                        nc.vector.tensor_add(aw[:], aw[:], b1_ap)
                        nc.vector.tensor_scalar_max(h_sb[:, mb, :], aw[:],
                                                    0.0)
                        tp = apool.tile([P, P], bf16, space="PSUM",
                                        tag="auxb")
                        nc.tensor.transpose(tp[:16, :], h_sb[:, mb, :],
                                            identb)
                        nc.vector.tensor_copy(
                            hT_sb[:, mb * P:(mb + 1) * P], tp[:16, :])
                    for mb in range(2):
                        psz = apool.tile([P, W_OUT], f32, space="PSUM",
                                         tag="aux")
                        nc.tensor.matmul(psz[:],
                                         hT_sb[:, mb * P:(mb + 1) * P],
                                         gw2_ap, start=True, stop=True)
                        nc.vector.tensor_scalar_mul(w2_sb[:, mb, :], psz[:],
                                                    dinv[:, mb, :])
                    rhs_e = allgather("E", w2_sb)
                    _stage("passD", rhs_e[:, 0, 0:1])

                    # ================ PASS E (GCN2 mats0) ================
                    shE = pool.tile([P, 2, C * W_OUT], bf16, tag="shE")
                    for c in range(C):
                        for mb in range(2):
                            ps = ppool.tile([P, W_OUT], f32, space="PSUM",
                                            tag="ep")
                            for kc in range(KC):
                                nc.tensor.matmul(
                                    ps[:], mchunk(0, c, kc, mb),
                                    rhs_e[:, kc, :],
                                    start=(kc == 0), stop=(kc == KC - 1))
                            nc.vector.tensor_copy(
                                shE[:, mb, 64 * c:64 * c + 64], ps[:])
                    rhs_f = allgather("F", shE)
                    _stage("passE", rhs_f[:, 0, 0:1])

                    # ===== PASS F (GCN2 mats1) + log_softmax + head =====
                    awb = pool.tile([P, 2, W_OUT], bf16, tag="awb")
                    awT_sb = pool.tile([W_OUT, NS], bf16, tag="awT")
                    tot_sb = pool.tile([P, 2, 1], f32, tag="tot")
                    for mb in range(2):
                        ps = ppool.tile([P, W_OUT], f32, space="PSUM",
                                        tag="ep")
                        first = True
                        for c in range(C):
                            for kc in range(KC):
                                nc.tensor.matmul(
                                    ps[:], mchunk(1, c, kc, mb),
                                    rhs_f[:, kc, 64 * c:64 * c + 64],
                                    start=first,
                                    stop=(c == C - 1 and kc == KC - 1))
                                first = False
                        aw = pool.tile([P, W_OUT], f32, tag="aw2", bufs=2)
                        nc.vector.tensor_add(aw[:], ps[:], w2_sb[:, mb, :])
                        nc.vector.tensor_scalar_mul(aw[:], aw[:],
                                                    dinv[:, mb, :])
                        nc.vector.tensor_add(aw[:], aw[:], b2_ap)
                        mx = pool.tile([P, 1], f32, tag="mx", bufs=2)
                        nc.vector.tensor_reduce(mx[:], aw[:],
                                                axis=mybir.AxisListType.X,
                                                op=OP.max)
                        nmx = pool.tile([P, 1], f32, tag="nmx", bufs=2)
                        nc.vector.tensor_scalar_mul(nmx[:], mx[:], -1.0)
                        ee = pool.tile([P, W_OUT], f32, tag="ee", bufs=2)
                        nc.scalar.activation(ee[:], aw[:], AF.Exp,
                                             bias=nmx[:, :])
                        ssum = pool.tile([P, 1], f32, tag="ssum", bufs=2)
                        nc.vector.tensor_reduce(ssum[:], ee[:],
                                                axis=mybir.AxisListType.X,
                                                op=OP.add)
                        lns = pool.tile([P, 1], f32, tag="lns", bufs=2)
                        nc.scalar.activation(lns[:], ssum[:], AF.Ln)
                        nc.vector.tensor_add(tot_sb[:, mb, :], mx[:],
                                             lns[:])
                        nc.vector.tensor_copy(awb[:, mb, :], aw[:])
                        tp = apool.tile([P, P], bf16, space="PSUM",
                                        tag="auxb")
                        nc.tensor.transpose(tp[:W_OUT, :], awb[:, mb, :],
                                            identb)
                        nc.vector.tensor_copy(
                            awT_sb[:, mb * P:(mb + 1) * P], tp[:W_OUT, :])
                    # z = aw @ lw - tot * colsum(lw)  (per local shard)
                    z_sb = pool.tile([P, 2, NCLS], bf16, tag="z")
                    for mb in range(2):
                        psz = apool.tile([P, NCLS], f32, space="PSUM",
                                         tag="aux")
                        nc.tensor.matmul(psz[:],
                                         awT_sb[:, mb * P:(mb + 1) * P],
                                         lw_ap, start=True, stop=True)
                        nc.vector.scalar_tensor_tensor(
                            out=z_sb[:, mb, :], in0=csn_ap,
                            scalar=tot_sb[:, mb, :], in1=psz[:],
                            op0=OP.mult, op1=OP.add)
                    # y rows owned by this core: one-hot select + bias
                    psy = apool.tile([P, NCLS], f32, space="PSUM", tag="aux")
                    nc.tensor.matmul(psy[:], sel0_ap, z_sb[:, 0, :],
                                     start=True, stop=False)
                    nc.tensor.matmul(psy[:], sel1_ap, z_sb[:, 1, :],
                                     start=False, stop=True)
                    y_sb = pool.tile([P, NCLS], f32, tag="y_sb")
                    nc.vector.tensor_add(y_sb[:], psy[:], lb_ap)
                    nc.gpsimd.indirect_dma_start(
                        out=y_d[:], out_offset=bass.IndirectOffsetOnAxis(
                            ap=yidx_ap[:, 0:1], axis=0),
                        in_=y_sb[:], in_offset=None)
                except _StageStop:
                    lt = stage_state["last"]
                    y_sb = pool.tile([P, NCLS], f32, tag="y_sb")
                    nc.vector.memset(y_sb[:], 0.0)
                    nc.vector.tensor_scalar_mul(y_sb[:, 0:1], lt, 0.0)
                    yv = y_d[:].rearrange("(b p) n -> b p n", p=P)
                    for b in range(NTGT // P):
                        nc.sync.dma_start(yv[b], y_sb[:, 0:NCLS])
                prev_y = y_sb[:, 0:1]

    nc.compile()
    return nc


# ---------------------------------------------------------------------------
# Execution via PJRT (axon) with a persistent jitted callable.
# ---------------------------------------------------------------------------
class _Runner:
    def __init__(self, nc, n_cores):
        import jax
        from jax.sharding import Mesh, PartitionSpec
        from jax.experimental.shard_map import shard_map
        from concourse.bass2jax import (
            _bass_exec_p, install_neuronx_cc_hook, partition_id_tensor)

        install_neuronx_cc_hook()
        self.jax = jax
        self._nc = nc
        self.n_cores = n_cores
        partition_name = (
            nc.partition_id_tensor.name if nc.partition_id_tensor else None)
        in_names, out_names, out_avals, zero_outs = [], [], [], []
        for alloc in nc.m.functions[0].allocations:
            if not isinstance(alloc, mybir.MemoryLocationSet):
                continue
            name = alloc.memorylocations[0].name
            if alloc.kind == "ExternalInput":
                if name != partition_name:
                    in_names.append(name)
            elif alloc.kind == "ExternalOutput":
                shape = tuple(alloc.tensor_shape)
                dtype = mybir.dt.np(alloc.dtype)
                out_names.append(name)
                out_avals.append(jax.core.ShapedArray(shape, dtype))
                zero_outs.append(np.zeros(shape, dtype))
        self.n_params = len(in_names)
        self.out_names = out_names
        self.out_avals = out_avals
        self.zero_outs = zero_outs
        n_outs = len(out_avals)
        in_names = in_names + out_names
        if partition_name is not None:
            in_names.append(partition_name)
        self.in_names = in_names

        def _body(*args):
            operands = list(args)
            if partition_name is not None:
                operands.append(partition_id_tensor())
            outs = _bass_exec_p.bind(
                *operands, out_avals=tuple(out_avals),
                in_names=tuple(in_names), out_names=tuple(out_names),
                lowering_input_output_aliases=(),
                sim_require_finite=True, sim_require_nnan=True, nc=nc)
            return tuple(outs)

        devices = jax.devices()[:n_cores]
        mesh = Mesh(np.asarray(devices), ("core",))
        in_specs = (PartitionSpec("core"),) * (self.n_params + n_outs)
        out_specs = (PartitionSpec("core"),) * n_outs
        self._fn = jax.jit(
            shard_map(_body, mesh=mesh, in_specs=in_specs,
                      out_specs=out_specs, check_rep=False),
            donate_argnums=tuple(range(self.n_params,
                                       self.n_params + n_outs)),
            keep_unused=True)

    def concat_inputs(self, in_maps):
        return [
            np.concatenate([np.asarray(m[name]) for m in in_maps], axis=0)
            for name in self.in_names[: self.n_params]
        ]

    def zeros(self):
        return [
            np.zeros((self.n_cores * z.shape[0], *z.shape[1:]), z.dtype)
            for z in self.zero_outs
        ]

    def run(self, in_maps):
        outs = self._fn(*self.concat_inputs(in_maps), *self.zeros())
        return [
            {
                name: np.asarray(outs[i]).reshape(
                    self.n_cores, *self.out_avals[i].shape)[c]
                for i, name in enumerate(self.out_names)
            }
            for c in range(self.n_cores)
        ]


_CACHE = {}


def _get_runner(NI, M, scale, DUPW=0, reps=1, stop_after=None):
    key = (NI, M, scale, DUPW, reps, stop_after)
    if key not in _CACHE:
        nc = build_kernel(NI, M, scale, DUPW=DUPW, reps=reps,
                          stop_after=stop_after)
        _CACHE[key] = _Runner(nc, NCORES)
    return _CACHE[key]


def kernel(**inputs) -> np.ndarray:
    in_maps, NI, M, scale, DUPW = _prep_inputs(**inputs)
    runner = _get_runner(NI, M, scale, DUPW)
    results = runner.run(in_maps)
    # each core scatters only the y rows it owns; rows are disjoint
    return np.sum([results[k]["y"] for k in range(NCORES)], axis=0)
